# revision 5
# baseline (speedup 1.0000x reference)
"""Trainium2 Bass kernel for nn_BatchedSRNNCell.

Strategy: one ablation variant k per NeuronCore (K=8 == n_cores), feature-major
layout on-chip (partition = neuron index, free = batch).

Math restructuring (validated vs fp64 reference in numpy):
  * x-state kept shifted: x~ = x - a0' with a0' = a_0 + sum_ma c_0; the bias
    folds into in_cur (q = in_cur - a0').
  * semi-implicit Euler updates become x~' = cx1*(W@s + q + x~/gx) where
    cx1 = gx/(1+gx) (single ACT pass off PSUM).
  * adaptation state a_ma(t) = cA^t * (a0_ma + h_ma(t)); h update is a pure
    multiply-accumulate h += beta(t)*r with per-partition beta = cB/cA^(t+1).
    a0 stored fp16, its fp32 residual seeds h (precision preserved).
  * thresholds: ds = x~ - sum_ma cA^t (a0_ma + h_ma) built on the tensor engine
    with per-(ma,t) diagonal fp16 weight matrices accumulating in PSUM.
  * piecewise sigmooid via clamp identity:
       r = 5*c1^2 + c2s - 5*(c3s-x4)^2 + 0.5
    with c1 = clamp(ds-x1,0,.1), c2s = clamp(ds,x2,x3), c3s = clamp(ds,x3,x4);
    clamps are single dual-op tensor_scalar ops, squares ride the ACT engine,
    the 4-term combine rides the tensor engine (identity matmuls into PSUM).
  * b update: divisor inv tracked across steps with one fp16 Newton iteration
    per step (3 iterations at t=0 from a per-partition seed).
"""

import numpy as np

K, N, NE, NI, MA, IN, B = 8, 512, 256, 256, 3, 128, 1024
H, UNFOLDS = 0.04, 6
DT = H / UNFOLDS
STATE = NE * MA + NI * MA + NE + NI + N
NCHUNK = 4  # 4 partition chunks of 128 over N=512
SQ5 = float(np.sqrt(5.0))
X1, X2, X3, X4 = -0.55, -0.45, 0.45, 0.55

_CACHE = {}


# --------------------------------------------------------------------------
# vecs packing (host): (128, 53) per-partition parameter columns
#  0..3   log_tau_d chunk c
#  4+idx  log_tau_a   (idx = g*6 + ma*2 + cc)
#  16+idx log_c
#  28+g*2+cc log_tau_b_rec
#  32+g*2+cc log_tau_b_rel
#  36+c   a_0 chunk c
#  40+idx c_0
#  52     readout_id
NVIN = 53


def _pack_vecs(inputs, k):
    v = np.zeros((128, NVIN), np.float32)
    lt = np.asarray(inputs["log_tau_d"][k])
    a0 = np.asarray(inputs["a_0"][k])
    for c in range(4):
        v[:, c] = lt[c * 128:(c + 1) * 128]
        v[:, 36 + c] = a0[c * 128:(c + 1) * 128]
    for g, (lta, lc, c0) in enumerate([
        (inputs["log_tau_a_E"][k], inputs["log_c_E"][k], inputs["c_0_E"][k]),
        (inputs["log_tau_a_I"][k], inputs["log_c_I"][k], inputs["c_0_I"][k]),
    ]):
        lta = np.asarray(lta); lc = np.asarray(lc); c0 = np.asarray(c0)
        for ma in range(MA):
            for cc in range(2):
                idx = g * 6 + ma * 2 + cc
                sl = slice(cc * 128, (cc + 1) * 128)
                v[:, 4 + idx] = lta[sl, ma]
                v[:, 16 + idx] = lc[sl, ma]
                v[:, 40 + idx] = c0[sl, ma]
    for g, (rec, rel) in enumerate([
        (inputs["log_tau_b_rec_E"][k], inputs["log_tau_b_rel_E"][k]),
        (inputs["log_tau_b_rec_I"][k], inputs["log_tau_b_rel_I"][k]),
    ]):
        rec = np.asarray(rec); rel = np.asarray(rel)
        for cc in range(2):
            v[:, 28 + g * 2 + cc] = rec[cc * 128:(cc + 1) * 128]
            v[:, 32 + g * 2 + cc] = rel[cc * 128:(cc + 1) * 128]
    v[:, 52] = float(np.asarray(inputs["readout_ids"][k]))
    return v


# dv derived-column map (device)
DV_SP = 0        # softplus(vecs[0:36])          -> 0..35
DV_ISP = 36      # 1/softplus                    -> 36..71
DV_G = 72        # DT/softplus                   -> 72..107 (gx@72+c, ga@76+idx,
                 #   grec@100+g*2+cc, grel@104+g*2+cc)
DV_GA1 = 108     # 1+ga                          12
DV_CA = 120      # cA = 1/(1+ga)                 12
DV_CB = 144      # cB = ga*cE*cA (132 scratch)   12
DV_ICA = 156     # 1/cA                          12
DV_GX1 = 168     # 1+gx (4) ; cx0@172 (4); cx1@176 (4); gxinv@180 (4)
DV_GRECP1 = 184  # 1+grec (4)
DV_BETA = 188    # beta_t, t=0..5, 12 each       -> 188..259
DV_POW = 260     # cA^t, t=2..6, 12 each         -> 260..319  (t=1 -> DV_CA)
DV_A0P = 320     # a0' (4); na0'@324 (4)
DV_SEED = 328    # scratch 328..335, seed@336 (4)
DV_SEL = 340     # alpha,beta,gamma readout masks (3)
DVW = 344


def _build_nc(dt_knobs=None):
    import concourse.bass as bass
    import concourse.tile as tile
    from concourse import bacc, mybir
    from concourse.masks import make_identity

    f32 = mybir.dt.float32
    f32r = mybir.dt.float32r
    f16 = mybir.dt.float16
    ALU = mybir.AluOpType
    ACT = mybir.ActivationFunctionType

    nc = bacc.Bacc("TRN2", target_bir_lowering=False, debug=False, num_devices=K)

    # ---- DRAM I/O ----
    d_aE0 = nc.declare_dram_parameter("aE0", [MA * NE, B], f32, isOutput=False)
    d_aI0 = nc.declare_dram_parameter("aI0", [MA * NI, B], f32, isOutput=False)
    d_b0 = nc.declare_dram_parameter("b0", [N, B], f32, isOutput=False)
    d_x0 = nc.declare_dram_parameter("x0", [N, B], f32, isOutput=False)
    d_wT = nc.declare_dram_parameter("wT", [N, N], f32, isOutput=False)
    d_mT = nc.declare_dram_parameter("mT", [N, N], f32, isOutput=False)
    d_winT = nc.declare_dram_parameter("winT", [IN, N], f32, isOutput=False)
    d_inpT = nc.declare_dram_parameter("inpT", [IN, B], f32, isOutput=False)
    d_vecs = nc.declare_dram_parameter("vecs", [128, NVIN], f32, isOutput=False)

    d_o = nc.declare_dram_parameter("o_out", [N, B], f32, isOutput=True)
    d_x = nc.declare_dram_parameter("x_out", [N, B], f32, isOutput=True)
    d_bo = nc.declare_dram_parameter("b_out", [N, B], f32, isOutput=True)
    d_aEo = nc.declare_dram_parameter("aE_out", [MA * NE, B], f32, isOutput=True)
    d_aIo = nc.declare_dram_parameter("aI_out", [MA * NI, B], f32, isOutput=True)

    with tile.TileContext(nc) as tc:
        import contextlib
        with contextlib.ExitStack() as ctx:
            cpool = ctx.enter_context(tc.tile_pool(name="const", bufs=1))
            spool = ctx.enter_context(tc.tile_pool(name="state", bufs=1))
            wpool = ctx.enter_context(tc.tile_pool(name="work", bufs=2))
            w1pool = ctx.enter_context(tc.tile_pool(name="work1", bufs=1))
            p_ds = ctx.enter_context(tc.tile_pool(name="pds", bufs=2, space="PSUM"))
            p_r = ctx.enter_context(tc.tile_pool(name="pr", bufs=1, space="PSUM"))
            p_syn = ctx.enter_context(tc.tile_pool(name="psyn", bufs=1, space="PSUM"))

            # ---------- constants / identities ----------
            I32 = cpool.tile([128, 128], f32, tag="I32")
            make_identity(nc, I32[:])
            I16 = cpool.tile([128, 128], f16, tag="I16")
            nc.vector.tensor_copy(I16[:], I32[:])
            nI16 = cpool.tile([128, 128], f16, tag="nI16")
            nc.vector.tensor_scalar_mul(nI16[:], I16[:], -1.0)
            halfrow = cpool.tile([1, 128], f16, tag="halfrow")
            nc.vector.memset(halfrow[:], 0.5)
            onesrow = cpool.tile([1, 512], f16, tag="onesrow")
            nc.vector.memset(onesrow[:], 1.0)
            bias1 = cpool.tile([128, 1], f32, tag="bias1")
            nc.vector.memset(bias1[:], -SQ5 * X1)
            bias3 = cpool.tile([128, 1], f32, tag="bias3")
            nc.vector.memset(bias3[:], -SQ5 * X4)

            # ---------- vecs -> derived dv ----------
            vecs = cpool.tile([128, NVIN], f32, tag="vecs")
            nc.sync.dma_start(vecs[:], d_vecs[:])
            dv = cpool.tile([128, DVW], f32, tag="dv")

            def col(i, n=1):
                return dv[:, i:i + n]

            # softplus(x) = ln(1+exp(x))
            nc.scalar.activation(col(DV_SP, 36), vecs[:, 0:36], ACT.Exp)
            nc.vector.tensor_scalar_add(col(DV_SP, 36), col(DV_SP, 36), 1.0)
            nc.scalar.activation(col(DV_SP, 36), col(DV_SP, 36), ACT.Ln)
            nc.vector.reciprocal(col(DV_ISP, 36), col(DV_SP, 36))
            nc.vector.tensor_scalar_mul(col(DV_G, 36), col(DV_ISP, 36), DT)
            nc.vector.tensor_scalar_add(col(DV_GA1, 12), col(DV_G + 4, 12), 1.0)
            nc.vector.reciprocal(col(DV_CA, 12), col(DV_GA1, 12))
            nc.vector.tensor_mul(col(132, 12), col(DV_G + 4, 12), col(16, 12))
            nc.vector.tensor_mul(col(DV_CB, 12), col(132, 12), col(DV_CA, 12))
            nc.vector.reciprocal(col(DV_ICA, 12), col(DV_CA, 12))
            nc.vector.tensor_scalar_add(col(DV_GX1, 4), col(DV_G, 4), 1.0)
            nc.vector.reciprocal(col(172, 4), col(DV_GX1, 4))
            nc.vector.tensor_mul(col(176, 4), col(DV_G, 4), col(172, 4))
            nc.vector.tensor_scalar_mul(col(180, 4), col(DV_SP, 4), 1.0 / DT)
            nc.vector.tensor_scalar_add(col(DV_GRECP1, 4), col(DV_G + 28, 4), 1.0)
            # betas
            nc.vector.tensor_mul(col(DV_BETA, 12), col(DV_CB, 12), col(DV_ICA, 12))
            for t in range(1, 6):
                nc.vector.tensor_mul(col(DV_BETA + 12 * t, 12),
                                     col(DV_BETA + 12 * (t - 1), 12), col(DV_ICA, 12))
            # powers cA^t, t=2..6
            nc.vector.tensor_mul(col(DV_POW, 12), col(DV_CA, 12), col(DV_CA, 12))
            for t in range(3, 7):
                nc.vector.tensor_mul(col(DV_POW + 12 * (t - 2), 12),
                                     col(DV_POW + 12 * (t - 3), 12), col(DV_CA, 12))
            # a0' per x-chunk
            for c in range(4):
                g, cc = c // 2, c % 2
                nc.vector.tensor_add(col(DV_A0P + c), vecs[:, 36 + c:37 + c],
                                     vecs[:, 40 + g * 6 + 0 * 2 + cc:41 + g * 6 + cc])
                nc.vector.tensor_add(col(DV_A0P + c), col(DV_A0P + c),
                                     vecs[:, 40 + g * 6 + 1 * 2 + cc:41 + g * 6 + 2 + cc])
                nc.vector.tensor_add(col(DV_A0P + c), col(DV_A0P + c),
                                     vecs[:, 40 + g * 6 + 2 * 2 + cc:41 + g * 6 + 4 + cc])
            nc.vector.tensor_scalar_mul(col(DV_A0P + 4, 4), col(DV_A0P, 4), -1.0)
            # newton seed = 1/(1+grec+grel/2)
            nc.vector.tensor_scalar_mul(col(DV_SEED, 4), col(DV_G + 32, 4), 0.5)
            nc.vector.tensor_add(col(DV_SEED + 4, 4), col(DV_SEED, 4), col(DV_GRECP1, 4))
            nc.vector.reciprocal(col(336, 4), col(DV_SEED + 4, 4))
            # readout select masks
            for j, val in enumerate([0.0, 1.0, 2.0]):
                nc.vector.tensor_scalar(col(DV_SEL + j), vecs[:, 52:53], val, None,
                                        ALU.is_equal)

            def pow_ap(g, ma, cc, t):
                idx = g * 6 + ma * 2 + cc
                if t == 1:
                    return col(DV_CA + idx)
                return col(DV_POW + 12 * (t - 2) + idx)

            def beta_ap(g, ma, cc, t):
                return col(DV_BETA + 12 * t + g * 6 + ma * 2 + cc)

            # ---------- fp16 diagonal weight sets ----------
            diags = {}
            for g in range(2):
                for ma in range(MA):
                    for cc in range(2):
                        for t in range(1, 7):
                            dtile = cpool.tile([128, 128], f16,
                                               tag=f"dg{g}{ma}{cc}{t}")
                            nc.vector.tensor_scalar_mul(dtile[:], I16[:],
                                                        pow_ap(g, ma, cc, t))
                            diags[(g, ma, cc, t)] = dtile
            dgGX = []
            for c in range(4):
                dtile = cpool.tile([128, 128], f16, tag=f"dgx{c}")
                nc.vector.tensor_scalar_mul(dtile[:], I16[:], col(180 + c))
                dgGX.append(dtile)

            # ---------- W / W_in / inputs prep ----------
            w16 = []
            for mc in range(4):
                wt = wpool.tile([128, N], f32, tag="tail32")
                nc.sync.dma_start(wt[:], d_wT[mc * 128:(mc + 1) * 128, :])
                mt = wpool.tile([128, N], f32, tag="mld")
                nc.sync.dma_start(mt[:], d_mT[mc * 128:(mc + 1) * 128, :])
                nc.scalar.activation(wt[:], wt[:], ACT.Exp)
                nc.vector.tensor_scalar_add(wt[:], wt[:], 1.0)
                nc.scalar.activation(wt[:], wt[:], ACT.Ln)
                w = cpool.tile([128, N], f16, tag=f"w16_{mc}")
                nc.vector.tensor_mul(w[:], wt[:], mt[:])
                w16.append(w)
            win16 = cpool.tile([IN, N], f16, tag="win16")
            wt = wpool.tile([IN, N], f32, tag="tail32")
            nc.sync.dma_start(wt[:], d_winT[:])
            nc.vector.tensor_copy(win16[:], wt[:])
            inp16 = cpool.tile([IN, B], f16, tag="inp16")
            it = wpool.tile([IN, B], f32, tag="tail32")
            nc.sync.dma_start(it[:], d_inpT[:])
            nc.vector.tensor_copy(inp16[:], it[:])

            # in_cur -> q = in_cur - a0'
            q = []
            for c in range(4):
                pq = p_syn.tile([128, B], f32, tag="syn")
                for h in range(2):
                    nc.tensor.matmul(pq[:, h * 512:(h + 1) * 512],
                                     win16[:, c * 128:(c + 1) * 128],
                                     inp16[:, h * 512:(h + 1) * 512],
                                     start=True, stop=True)
                qc = cpool.tile([128, B], f16, tag=f"q{c}")
                nc.scalar.activation(qc[:], pq[:], ACT.Identity,
                                     bias=col(DV_A0P + 4 + c), scale=1.0)
                q.append(qc)

            # ---------- state load ----------
            xs = []
            for c in range(4):
                xc = spool.tile([128, B], f32, tag=f"x{c}")
                nc.sync.dma_start(xc[:], d_x0[c * 128:(c + 1) * 128, :])
                nc.scalar.activation(xc[:], xc[:], ACT.Identity,
                                     bias=col(DV_A0P + 4 + c), scale=1.0)
                xs.append(xc)
            b16 = []
            for c in range(4):
                bt = wpool.tile([128, B], f32, tag="tail32")
                nc.sync.dma_start(bt[:], d_b0[c * 128:(c + 1) * 128, :])
                bc = spool.tile([128, B], f16, tag=f"b{c}")
                nc.scalar.copy(bc[:], bt[:])
                b16.append(bc)
            a16 = {}
            hh = {}
            for g, dram in enumerate([d_aE0, d_aI0]):
                for ma in range(MA):
                    for cc in range(2):
                        row = ma * 256 + cc * 128
                        at = wpool.tile([128, B], f32, tag="tail32")
                        nc.sync.dma_start(at[:], dram[row:row + 128, :])
                        a1 = spool.tile([128, B], f16, tag=f"a16_{g}{ma}{cc}")
                        nc.scalar.copy(a1[:], at[:])
                        hcur = spool.tile([128, B], f16, tag=f"h{g}{ma}{cc}")
                        nc.vector.tensor_sub(hcur[:], at[:], a1[:])
                        a16[(g, ma, cc)] = a1
                        hh[(g, ma, cc)] = hcur
            inv16 = []
            for c in range(4):
                ic = spool.tile([128, B], f16, tag=f"inv{c}")
                nc.vector.memset(ic[:], 1.0)
                nc.vector.tensor_scalar_mul(ic[:], ic[:], col(336 + c))
                inv16.append(ic)

            grec_ap = [col(DV_G + 28 + (c // 2) * 2 + (c % 2)) for c in range(4)]
            grel_ap = [col(DV_G + 32 + (c // 2) * 2 + (c % 2)) for c in range(4)]
            grecp1_ap = [col(DV_GRECP1 + c) for c in range(4)]
            cx1_ap = [col(176 + c) for c in range(4)]
            a0p_ap = [col(DV_A0P + c) for c in range(4)]

            # ---------- main loop ----------
            for t in range(UNFOLDS + 1):
                final = t == UNFOLDS
                r16s, s16s, x16s = [], [], []
                for c in range(4):
                    g, cc = c // 2, c % 2
                    # ds accumulation: psum = thr - x~ ; ds = -psum
                    x16 = w1pool.tile([128, B], f16, tag=f"x16_{c}")
                    nc.scalar.copy(x16[:], xs[c][:])
                    x16s.append(x16)
                    pd = p_ds.tile([128, B], f32, tag="ds")
                    for h in range(2):
                        sl = slice(h * 512, (h + 1) * 512)
                        nc.tensor.matmul(pd[:, sl], nI16[:], x16[:, sl],
                                         start=True, stop=False)
                        for ma in range(MA):
                            lhs = I16[:] if t == 0 else diags[(g, ma, cc, t)][:]
                            nc.tensor.matmul(pd[:, sl], lhs, a16[(g, ma, cc)][:, sl],
                                             start=False, stop=(t == 0 and ma == MA - 1))
                        if t > 0:
                            for ma in range(MA):
                                nc.tensor.matmul(pd[:, sl], diags[(g, ma, cc, t)][:],
                                                 hh[(g, ma, cc)][:, sl],
                                                 start=False, stop=(ma == MA - 1))
                    ds = wpool.tile([128, B], f16, tag="ds")
                    nc.scalar.activation(ds[:], pd[:], ACT.Copy, scale=-1.0)
                    c1s = wpool.tile([128, B], f16, tag="c1s")
                    nc.vector.tensor_scalar(c1s[:], ds[:], X1, X2, ALU.max, ALU.min)
                    c2s = wpool.tile([128, B], f16, tag="c2s")
                    nc.vector.tensor_scalar(c2s[:], ds[:], X2, X3, ALU.max, ALU.min)
                    c3s = wpool.tile([128, B], f16, tag="c3s")
                    nc.vector.tensor_scalar(c3s[:], ds[:], X3, X4, ALU.max, ALU.min)
                    sq1 = wpool.tile([128, B], f16, tag="sq1")
                    nc.scalar.activation(sq1[:], c1s[:], ACT.Square,
                                         bias=bias1[:], scale=SQ5)
                    sq3 = wpool.tile([128, B], f16, tag="sq3")
                    nc.scalar.activation(sq3[:], c3s[:], ACT.Square,
                                         bias=bias3[:], scale=SQ5)
                    pr = p_r.tile([128, B], f32, tag="r")
                    for h in range(2):
                        sl = slice(h * 512, (h + 1) * 512)
                        nc.tensor.matmul(pr[:, sl], I16[:], sq1[:, sl],
                                         start=True, stop=False)
                        nc.tensor.matmul(pr[:, sl], I16[:], c2s[:, sl],
                                         start=False, stop=False)
                        nc.tensor.matmul(pr[:, sl], nI16[:], sq3[:, sl],
                                         start=False, stop=False)
                        nc.tensor.matmul(pr[:, sl], halfrow[:], onesrow[:],
                                         start=False, stop=True)
                    r16 = w1pool.tile([128, B], f16, tag=f"r16_{c}")
                    nc.scalar.copy(r16[:], pr[:])
                    s16 = w1pool.tile([128, B], f16, tag=f"s16_{c}")
                    nc.vector.tensor_mul(s16[:], r16[:], b16[c][:])
                    r16s.append(r16)
                    s16s.append(s16)

                    if not final:
                        # b update: inv via newton, b = (b+grec)*inv
                        den = wpool.tile([128, B], f16, tag="den")
                        nc.vector.tensor_scalar(den[:], r16[:], grel_ap[c],
                                                grecp1_ap[c], ALU.mult, ALU.add)
                        for _ in range(3 if t == 0 else 1):
                            tt = wpool.tile([128, B], f16, tag="tmpa")
                            nc.vector.tensor_mul(tt[:], den[:], inv16[c][:])
                            t2 = wpool.tile([128, B], f16, tag="tmpb")
                            nc.vector.tensor_scalar(t2[:], tt[:], -1.0, 2.0,
                                                    ALU.mult, ALU.add)
                            nc.vector.tensor_mul(inv16[c][:], inv16[c][:], t2[:])
                        nc.vector.scalar_tensor_tensor(
                            b16[c][:], b16[c][:], grec_ap[c], inv16[c][:],
                            ALU.add, ALU.mult)
                        # h updates
                        for ma in range(MA):
                            nc.vector.scalar_tensor_tensor(
                                hh[(g, ma, cc)][:], r16[:], beta_ap(g, ma, cc, t),
                                hh[(g, ma, cc)][:], ALU.mult, ALU.add)

                if not final:
                    for c in range(4):
                        ps = p_syn.tile([128, B], f32, tag="syn")
                        for h in range(2):
                            sl = slice(h * 512, (h + 1) * 512)
                            for mc in range(4):
                                nc.tensor.matmul(ps[:, sl],
                                                 w16[mc][:, c * 128:(c + 1) * 128],
                                                 s16s[mc][:, sl],
                                                 start=(mc == 0), stop=False)
                            nc.tensor.matmul(ps[:, sl], I16[:], q[c][:, sl],
                                             start=False, stop=False)
                            nc.tensor.matmul(ps[:, sl], dgGX[c][:],
                                             x16s[c][:, sl],
                                             start=False, stop=True)
                        nc.scalar.activation(xs[c][:], ps[:], ACT.Copy,
                                             scale=cx1_ap[c])
                else:
                    # ---------- outputs ----------
                    for c in range(4):
                        xt = wpool.tile([128, B], f32, tag="tail32")
                        nc.vector.tensor_scalar_add(xt[:], xs[c][:], a0p_ap[c])
                        nc.sync.dma_start(d_x[c * 128:(c + 1) * 128, :], xt[:])
                        o16 = wpool.tile([128, B], f16, tag="o16")
                        nc.vector.tensor_scalar_mul(o16[:], s16s[c][:], col(DV_SEL))
                        nc.vector.scalar_tensor_tensor(
                            o16[:], r16s[c][:], col(DV_SEL + 1), o16[:],
                            ALU.mult, ALU.add)
                        o32 = wpool.tile([128, B], f32, tag="tail32")
                        nc.vector.scalar_tensor_tensor(
                            o32[:], xt[:], col(DV_SEL + 2), o16[:],
                            ALU.mult, ALU.add)
                        nc.sync.dma_start(d_o[c * 128:(c + 1) * 128, :], o32[:])
                        bo = wpool.tile([128, B], f32, tag="tail32")
                        nc.scalar.copy(bo[:], b16[c][:])
                        nc.sync.dma_start(d_bo[c * 128:(c + 1) * 128, :], bo[:])
                    # adaptation state reconstruction on PE
                    for i, (g, dram) in enumerate([(0, d_aEo), (1, d_aIo)]):
                        for ma in range(MA):
                            for cc in range(2):
                                row = ma * 256 + cc * 128
                                pa = (p_syn if (ma + cc) % 2 == 0 else p_r).tile(
                                    [128, B], f32, tag="syn" if (ma + cc) % 2 == 0 else "r")
                                dg = diags[(g, ma, cc, 6)]
                                for h in range(2):
                                    sl = slice(h * 512, (h + 1) * 512)
                                    nc.tensor.matmul(pa[:, sl], dg[:],
                                                     a16[(g, ma, cc)][:, sl],
                                                     start=True, stop=False)
                                    nc.tensor.matmul(pa[:, sl], dg[:],
                                                     hh[(g, ma, cc)][:, sl],
                                                     start=False, stop=True)
                                ao = wpool.tile([128, B], f32, tag="tail32")
                                nc.vector.tensor_copy(ao[:], pa[:])
                                nc.sync.dma_start(dram[row:row + 128, :], ao[:])

    nc.compile()
    return nc


def _host_prep(inputs):
    sgn = np.concatenate([np.ones(NE, np.float32), -np.ones(NI, np.float32)])
    inpT = np.ascontiguousarray(np.asarray(inputs["inputs"], np.float32).T)
    in_maps = []
    for k in range(K):
        st = np.asarray(inputs["state"][k], np.float32)
        i0 = NE * MA; i1 = i0 + NI * MA; i2 = i1 + NE; i3 = i2 + NI
        aE0 = np.ascontiguousarray(
            st[:, :i0].reshape(B, NE, MA).transpose(2, 1, 0).reshape(MA * NE, B))
        aI0 = np.ascontiguousarray(
            st[:, i0:i1].reshape(B, NI, MA).transpose(2, 1, 0).reshape(MA * NI, B))
        b0 = np.ascontiguousarray(st[:, i1:i3].T)
        x0 = np.ascontiguousarray(st[:, i3:].T)
        wT = np.ascontiguousarray(np.asarray(inputs["W_raw"][k], np.float32).T)
        mT = np.ascontiguousarray(
            np.asarray(inputs["sparsity_masks"][k], np.float32).T * sgn[:, None])
        winT = np.ascontiguousarray(np.asarray(inputs["W_in"][k], np.float32).T)
        in_maps.append({
            "aE0": aE0, "aI0": aI0, "b0": b0, "x0": x0, "wT": wT, "mT": mT,
            "winT": winT, "inpT": inpT, "vecs": _pack_vecs(inputs, k),
        })
    return in_maps


def _host_post(results):
    out = np.empty((K, B, N), np.float32)
    new_state = np.empty((K, B, STATE), np.float32)
    i0 = NE * MA; i1 = i0 + NI * MA; i2 = i1 + NE; i3 = i2 + NI
    for k in range(K):
        r = results[k]
        out[k] = r["o_out"].T
        new_state[k, :, :i0] = (
            r["aE_out"].reshape(MA, NE, B).transpose(2, 1, 0).reshape(B, MA * NE))
        new_state[k, :, i0:i1] = (
            r["aI_out"].reshape(MA, NI, B).transpose(2, 1, 0).reshape(B, MA * NI))
        new_state[k, :, i1:i3] = r["b_out"].T
        new_state[k, :, i3:] = r["x_out"].T
    return out, new_state


def get_nc():
    if "nc" not in _CACHE:
        _CACHE["nc"] = _build_nc()
    return _CACHE["nc"]


def kernel(**inputs):
    from concourse.bass_utils import run_bass_kernel_spmd
    nc = get_nc()
    in_maps = _host_prep(inputs)
    res = run_bass_kernel_spmd(nc, in_maps, list(range(K)))
    return _host_post(res.results)
